# revision 11
# baseline (speedup 1.0000x reference)
"""Trainium2 Bass kernel for the nn_Attention problem (B=4, S=1024, H=32, D=128).

Sharding: zero-collective data-parallel split. Each of the 8 cores owns one
(batch, half) pair: batch b = core//2, half = core%2. A half owns 4 of the 8
query blocks of 128 tokens, interleaved for causal balance:
  half 0 -> blocks [7, 5, 2, 0]   half 1 -> blocks [6, 4, 3, 1]
(both sum to 18 causal block-units, and the per-key-block column prefixes of
the two halves differ by at most one block, which makes the shared-program
NVMAX prefix sum optimal: 20 block-units vs 22 for the pairwise split).
Query columns are packed in DESCENDING block order so that key-block t only
needs a PREFIX of the packed columns. Each core computes Q proj (its tokens,
all heads), K/V proj (its whole batch), causal attention and the full output
projection for its tokens, then the host scatters the 8 token-slices back
into the full [B, S, HID] output.

Schedule (v4):
- Front is DMA-bandwidth-bound (~250 GB/s effective), so Q-proj inputs
  (hidq, wq0-3) are DMA'd first and heads 0-4 are projected while the KV
  inputs (wkv + 8 MB hid3) stream in; the KV loop then runs without stalls.
- Steady state emits, per iteration: attention for head h-5 in two 4-block
  phases with head-h Q-proj matmuls interleaved as PE filler, so the
  in-order engine streams never block the PE behind the softmax chain.
- The Q eviction + RoPE of head h are split across the iteration boundary:
  evict+swap-DMA at the start of iteration h+1 (Sync deps already met) and
  the RoPE vector ops LATE in iteration h+1, so the partition-swap DMA
  round-trip never stalls the DVE stream that feeds sums/exacc.
- Tail: the 5 drain heads' softmax chains are covered by partial O-proj
  accumulations for m=1,0 (two PSUM banks).

On-chip layout is transposed ([feature, token]) so every matmul has the
contraction dim on partitions with no transposes in the hot path.
"""

import numpy as np
import ml_dtypes

import concourse.bass as bass
import concourse.tile as tile
from concourse import bacc, mybir
from concourse.bass_utils import run_bass_kernel_spmd

B, S, H, D = 4, 1024, 32, 128
HID = H * D          # 4096
ROT = D // 2         # 64
HR = ROT // 2        # 32
ROPE_BASE = 10000.0
P = 128
NT = 512             # query tokens per core
NCORES = 8
KT = HID // P        # 32 contraction tiles
NKB = S // P         # 8 key blocks
SCALE = float(D) ** -0.5
PRE = 5              # heads projected before the KV phase (pipeline depth)

BLOCKS = [[7, 5, 2, 0], [6, 4, 3, 1]]
NVMAX = [512, 512, 384, 384, 256, 256, 128, 128]
MWIN = [(384, 512), (384, 512), (256, 384), (256, 384),
        (128, 256), (128, 256), (0, 128), (0, 128)]

BF = mybir.dt.bfloat16
F32 = mybir.dt.float32
AF = mybir.ActivationFunctionType
bf16 = ml_dtypes.bfloat16

_PROG = None


def _build():
    nc = bacc.Bacc("TRN2", target_bir_lowering=False, debug=False,
                   num_devices=NCORES)
    dp = nc.declare_dram_parameter
    hid3 = dp("hid3", [P, KT, S], BF, False)        # [p, k, t] = hidden[b,t,k*128+p]
    hidq = dp("hidq", [P, KT, NT], BF, False)       # packed query columns
    wq4 = dp("wq4", [H, P, KT, P], BF, False)       # [h,p,k,d] = Wq[h*128+d, k*128+p]
    wo4 = dp("wo4", [KT, P, KT, P], BF, False)      # [m,p,k,d] = Wo[m*128+d, k*128+p]
    wkv3 = dp("wkv3", [P, KT, 2 * D], BF, False)    # [p,k,c] = Wkv[c, k*128+p]
    bq2 = dp("bq2", [P, H], F32, False)
    bo2 = dp("bo2", [P, KT], F32, False)
    bkv2 = dp("bkv2", [P, 2], F32, False)
    cosq = dp("cosq", [ROT, NT], BF, False)
    sinq = dp("sinq", [ROT, NT], BF, False)
    cosk = dp("cosk", [ROT, S], BF, False)
    sink = dp("sink", [ROT, S], BF, False)
    maskt = dp("maskt", [P, NKB, P], BF, False)     # additive 0/-1e30, window only
    ident = dp("ident", [P, P], BF, False)
    outp = dp("out", [KT, P, NT], BF, True)        # [m, dd, c] = out.T slice

    with tile.TileContext(nc) as tc:
        with (
            tc.tile_pool(name="const", bufs=1) as constp,
            tc.tile_pool(name="persist", bufs=1) as persist,
            tc.tile_pool(name="wqp", bufs=4) as wqp,
            tc.tile_pool(name="qtmp", bufs=3) as qtmp,
            tc.tile_pool(name="qrp", bufs=6) as qrp,
            tc.tile_pool(name="expp", bufs=5) as expp,
            tc.tile_pool(name="exap", bufs=2) as exap,
            tc.tile_pool(name="nrm", bufs=2) as nrm,
            tc.tile_pool(name="outsb", bufs=2) as outsb,
        ):
            attn_all = persist.tile([P, KT, NT], BF, tag="attn")
            kbf = persist.tile([P, S], BF, tag="kbf")
            vnat = persist.tile([P, NKB, P], BF, tag="vnat")
            hidq_sb = persist.tile([P, KT, NT], BF, tag="hidq")

            # ---- earliest DMAs, interleaved so qproj(0) streams: the PE can
            # start on (hidq c1, wq0 c1) while the rest arrives ----
            nc.sync.dma_start(hidq_sb[:, 0:8, :], hidq[:, 0:8, :])
            wq_pre = {}
            w = wqp.tile([P, KT, P], BF, tag="w")
            nc.sync.dma_start(w[:, 0:8, :], wq4[0, :, 0:8, :])
            nc.sync.dma_start(w[:, 8:32, :], wq4[0, :, 8:32, :])
            wq_pre[0] = w
            for h in range(1, 4):
                nc.sync.dma_start(hidq_sb[:, 8 * h:8 * h + 8, :],
                                  hidq[:, 8 * h:8 * h + 8, :])
                w = wqp.tile([P, KT, P], BF, tag="w")
                nc.sync.dma_start(w[:], wq4[h])
                wq_pre[h] = w
            bq_sb = constp.tile([P, H], F32, tag="bq")
            nc.sync.dma_start(bq_sb[:], bq2[:])
            cosq_sb = constp.tile([ROT, NT], BF, tag="cq")
            nc.sync.dma_start(cosq_sb[:], cosq[:])
            sinq_sb = constp.tile([ROT, NT], BF, tag="sq")
            nc.sync.dma_start(sinq_sb[:], sinq[:])

            # warm up the ScalarE activation table (lazy 1.3us ACT_TABLE_LOAD
            # otherwise lands on the first eviction's critical path)
            scr = constp.tile([1, 1], F32, tag="scr")
            nc.gpsimd.memset(scr[:], 0.0)
            nc.scalar.activation(scr[:], scr[:], AF.Identity)

            with tc.tile_pool(name="psq", bufs=2, space="PSUM") as psq:

                def qproj_matmuls(qp, wq_sb, k0, k1):
                    for k in range(k0, k1):
                        nc.tensor.matmul(qp[:], wq_sb[:, k, :], hidq_sb[:, k, :],
                                         start=(k == 0), stop=(k == KT - 1))

                def qproj_evict(h, qp, swap_engine):
                    qfp = qtmp.tile([P, NT], BF, tag="qfp")
                    nc.scalar.activation(qfp[0:ROT, :], qp[0:ROT, :], AF.Identity,
                                         bias=bq_sb[0:ROT, h:h + 1])
                    qsw = qtmp.tile([ROT, NT], BF, tag="qsw")
                    swap_engine.dma_start(qsw[0:HR, :], qfp[HR:ROT, :])
                    swap_engine.dma_start(qsw[HR:ROT, :], qfp[0:HR, :])
                    nc.scalar.activation(qfp[ROT:P, :], qp[ROT:P, :], AF.Identity,
                                         bias=bq_sb[ROT:P, h:h + 1])
                    return qfp, qsw

                def qproj_rope(qfp, qsw):
                    m1q = qtmp.tile([ROT, NT], BF, tag="m1q")
                    m2q = qtmp.tile([ROT, NT], BF, tag="m2q")
                    qr = qrp.tile([P, NT], BF, tag="qr")
                    nc.vector.tensor_mul(m1q[:], qfp[0:ROT, :], cosq_sb[:])
                    nc.vector.tensor_mul(m2q[:], qsw[:], sinq_sb[:])
                    nc.vector.tensor_sub(qr[0:HR, :], m1q[0:HR, :], m2q[0:HR, :])
                    nc.vector.tensor_add(qr[HR:ROT, :], m1q[HR:ROT, :],
                                         m2q[HR:ROT, :])
                    nc.vector.tensor_copy(qr[ROT:P, :], qfp[ROT:P, :])
                    return qr

                def emit_qproj_front(h, wq_sb, pool):
                    qp = pool.tile([P, NT], F32, tag="qp")
                    qproj_matmuls(qp, wq_sb, 0, KT)
                    # front: swap via ScalarE so the Sync bulk stream is
                    # never head-of-line blocked waiting on an eviction
                    qfp, qsw = qproj_evict(h, qp, nc.scalar)
                    return qproj_rope(qfp, qsw)

                qrs = {}
                # dedicated 3-deep PSUM pool so qp(2) never waits on qp(0)'s
                # eviction; closes before the KV pools claim the banks
                with tc.tile_pool(name="fpq", bufs=3, space="PSUM") as fpq:
                    qrs[0] = emit_qproj_front(0, wq_pre[0], fpq)
                    qrs[1] = emit_qproj_front(1, wq_pre[1], fpq)
                    qrs[2] = emit_qproj_front(2, wq_pre[2], fpq)

                with (
                    tc.tile_pool(name="hidp", bufs=8) as hidp,
                    tc.tile_pool(name="wkvp", bufs=1) as wkvp,
                    tc.tile_pool(name="kvtmp", bufs=1) as kvtmp,
                    tc.tile_pool(name="pskv", bufs=1, space="PSUM") as pskv,
                    tc.tile_pool(name="pstr", bufs=2, space="PSUM") as pstr,
                ):
                    kps = [pskv.tile([P, S // 2], F32, name=f"kps{j}",
                                     tag=f"kps{j}") for j in range(2)]
                    vps = [pskv.tile([P, S // 2], F32, name=f"vps{j}",
                                     tag=f"vps{j}") for j in range(2)]
                    wkv_all = wkvp.tile([P, KT, 2 * D], BF, tag="wkvall")
                    nc.sync.dma_start(wkv_all[:, 0:8, :], wkv3[:, 0:8, :])
                    nc.sync.dma_start(wkv_all[:, 8:20, :], wkv3[:, 8:20, :])
                    nc.sync.dma_start(wkv_all[:, 20:32, :], wkv3[:, 20:32, :])

                    hh0 = hidp.tile([P, 1, S], BF, name="hh0", tag="hh")
                    nc.sync.dma_start(hh0[:], hid3[:, 0:1, :])
                    hhs = []
                    for g in range(15):
                        hh = hidp.tile([P, 2, S], BF, tag="hh")
                        nc.sync.dma_start(hh[:],
                                          hid3[:, 1 + 2 * g:3 + 2 * g, :])
                        hhs.append(hh)
                        if g == 5:
                            # first 8 hh tiles (7 issues) fit the pool without
                            # buffer waits; slot wq4/wq5 in before the paced rest
                            w = wqp.tile([P, KT, P], BF, tag="w")
                            nc.sync.dma_start(w[:], wq4[4])
                            wq_pre[4] = w
                            w = wqp.tile([P, KT, P], BF, tag="w")
                            nc.sync.dma_start(w[:], wq4[5])
                            wq_pre[5] = w
                    hh31 = hidp.tile([P, 1, S], BF, name="hh31", tag="hh")
                    nc.sync.dma_start(hh31[:], hid3[:, 31:32, :])

                    def hh_slice(k):
                        if k == 0:
                            return hh0[:, 0, :]
                        if k == KT - 1:
                            return hh31[:, 0, :]
                        return hhs[(k - 1) // 2][:, (k - 1) % 2, :]

                    cosk_sb = constp.tile([ROT, S], BF, tag="ck")
                    nc.sync.dma_start(cosk_sb[:], cosk[:])
                    sink_sb = constp.tile([ROT, S], BF, tag="sk")
                    nc.sync.dma_start(sink_sb[:], sink[:])
                    bkv_sb = constp.tile([P, 2], F32, tag="bkv")
                    nc.sync.dma_start(bkv_sb[:], bkv2[:])
                    id_sb = constp.tile([P, P], BF, tag="id")
                    nc.sync.dma_start(id_sb[:], ident[:])
                    mask_sb = constp.tile([P, NKB, P], BF, tag="mask")
                    nc.sync.dma_start(mask_sb[:], maskt[:])
                    bo_sb = constp.tile([P, KT], F32, tag="bo")
                    nc.sync.dma_start(bo_sb[:], bo2[:])
                    w = wqp.tile([P, KT, P], BF, tag="w")
                    nc.sync.dma_start(w[:], wq4[6])
                    wq_pre[6] = w
                    w = wqp.tile([P, KT, P], BF, tag="w")
                    nc.sync.dma_start(w[:], wq4[7])
                    wq_pre[7] = w
                    ones_sb = constp.tile([P, 1], BF, tag="ones")
                    nc.gpsimd.memset(ones_sb[:], 1.0)

                    qrs[3] = emit_qproj_front(3, wq_pre.pop(3), psq)
                    qrs[4] = emit_qproj_front(4, wq_pre.pop(4), psq)

                    # KV matmuls (kps pair shares a weight load, then vps pair)
                    for k in range(KT):
                        st, sp_ = (k == 0), (k == KT - 1)
                        hhk = hh_slice(k)
                        for j in range(2):
                            nc.tensor.matmul(kps[j][:], wkv_all[:, k, 0:D],
                                             hhk[:, j * 512:(j + 1) * 512],
                                             start=st, stop=sp_)
                        for j in range(2):
                            nc.tensor.matmul(vps[j][:], wkv_all[:, k, D:2 * D],
                                             hhk[:, j * 512:(j + 1) * 512],
                                             start=st, stop=sp_)

                    # ---- KV epilogue: evictions, K RoPE, V PE-transpose ----
                    kfp = kvtmp.tile([P, S], BF, tag="kfp")
                    vbf = kvtmp.tile([P, S], BF, tag="vbf")
                    for j in range(2):
                        sl = slice(j * 512, (j + 1) * 512)
                        nc.scalar.activation(kfp[:, sl], kps[j][:], AF.Identity,
                                             bias=bkv_sb[:, 0:1])
                        nc.scalar.activation(vbf[:, sl], vps[j][:], AF.Identity,
                                             bias=bkv_sb[:, 1:2])
                    ksw = kvtmp.tile([ROT, S], BF, tag="ksw")
                    nc.scalar.dma_start(ksw[0:HR, :], kfp[HR:ROT, :])
                    nc.scalar.dma_start(ksw[HR:ROT, :], kfp[0:HR, :])
                    m1 = kvtmp.tile([ROT, S], BF, tag="m1")
                    m2 = kvtmp.tile([ROT, S], BF, tag="m2")
                    nc.vector.tensor_mul(m1[:], kfp[0:ROT, :], cosk_sb[:])
                    nc.vector.tensor_mul(m2[:], ksw[:], sink_sb[:])
                    nc.vector.tensor_sub(kbf[0:HR, :], m1[0:HR, :], m2[0:HR, :])
                    nc.vector.tensor_add(kbf[HR:ROT, :], m1[HR:ROT, :],
                                         m2[HR:ROT, :])
                    nc.vector.tensor_copy(kbf[ROT:P, :], kfp[ROT:P, :])
                    for t in range(NKB):
                        vt = pstr.tile([P, P], BF, tag="vt")
                        nc.tensor.transpose(vt[:], vbf[:, t * P:(t + 1) * P],
                                            id_sb[:])
                        nc.vector.tensor_copy(vnat[:, t, :], vt[:])

                with (
                    tc.tile_pool(name="pss", bufs=4, space="PSUM") as pss,
                    tc.tile_pool(name="pso", bufs=1, space="PSUM") as pso,
                    tc.tile_pool(name="pssum", bufs=1, space="PSUM") as pssum,
                ):
                    def attn_scores(qr, trange):
                        sps = []
                        for t in trange:
                            nv = NVMAX[t]
                            sp = pss.tile([P, NT], F32, tag="sp")
                            nc.tensor.matmul(sp[:, 0:nv],
                                             kbf[:, t * P:(t + 1) * P],
                                             qr[:, 0:nv], start=True, stop=True)
                            sps.append((t, sp))
                        for t, sp in sps:
                            lo, hi = MWIN[t]
                            nc.vector.tensor_add(sp[:, lo:hi], sp[:, lo:hi],
                                                 mask_sb[:, t, :])
                        return sps

                    def attn_pv(sps, op):
                        exs = []
                        for t, sp in sps:
                            ex = expp.tile([P, NT], BF, tag="ex")
                            nc.scalar.activation(ex[:, 0:NVMAX[t]],
                                                 sp[:, 0:NVMAX[t]], AF.Exp,
                                                 scale=SCALE)
                            exs.append((t, ex))
                        for t, ex in exs:
                            nc.tensor.matmul(op[:, 0:NVMAX[t]], vnat[:, t, :],
                                             ex[:, 0:NVMAX[t]],
                                             start=(t == 0), stop=(t == NKB - 1))
                        return exs

                    def attn_exacc(exacc, exs):
                        for t, ex in exs:
                            if t == 0:
                                nc.vector.tensor_copy(exacc[:], ex[:])
                            else:
                                nc.vector.tensor_add(exacc[:, 0:NVMAX[t]],
                                                     exacc[:, 0:NVMAX[t]],
                                                     ex[:, 0:NVMAX[t]])

                    def attn_end(h, op, exacc):
                        sums = pssum.tile([1, NT], F32, tag="sums")
                        nc.tensor.matmul(sums[:], ones_sb[:], exacc[:],
                                         start=True, stop=True)
                        rs = nrm.tile([1, NT], F32, tag="rs")
                        nc.vector.reciprocal_approx_fast(rs[:], sums[:])
                        rb = nrm.tile([P, NT], F32, tag="rb")
                        nc.gpsimd.partition_broadcast(rb[:], rs[:])
                        nc.vector.tensor_mul(attn_all[:, h, :], op[:], rb[:])

                    def emit_attn(h, qr):
                        op = pso.tile([P, NT], F32, tag="op")
                        exacc = exap.tile([P, NT], BF, tag="exacc")
                        sps1 = attn_scores(qr, range(0, 4))
                        exs1 = attn_pv(sps1, op)
                        attn_exacc(exacc, exs1)
                        sps2 = attn_scores(qr, range(4, NKB))
                        exs2 = attn_pv(sps2, op)
                        attn_exacc(exacc, exs2)
                        attn_end(h, op, exacc)

                    # ---- steady-state head pipeline (depth PRE=5) ----
                    pending = None   # (h, qp) awaiting evict+rope
                    for h in range(PRE, H):
                        if h in wq_pre:
                            wq_sb = wq_pre.pop(h)
                        else:
                            wq_sb = wqp.tile([P, KT, P], BF, tag="w")
                            nc.sync.dma_start(wq_sb[:], wq4[h])
                        qp = psq.tile([P, NT], F32, tag="qp")
                        ha = h - PRE
                        qr_a = qrs.pop(ha)
                        op = pso.tile([P, NT], F32, tag="op")
                        exacc = exap.tile([P, NT], BF, tag="exacc")
                        if pending is not None:
                            hp, qpp = pending
                            ev = qproj_evict(hp, qpp, nc.sync)
                        sps1 = attn_scores(qr_a, range(0, 4))
                        qproj_matmuls(qp, wq_sb, 0, 6)
                        exs1 = attn_pv(sps1, op)
                        qproj_matmuls(qp, wq_sb, 6, 12)
                        attn_exacc(exacc, exs1)
                        sps2 = attn_scores(qr_a, range(4, NKB))
                        exs2 = attn_pv(sps2, op)
                        qproj_matmuls(qp, wq_sb, 12, 16)
                        attn_exacc(exacc, exs2)
                        attn_end(ha, op, exacc)
                        if pending is not None:
                            qrs[pending[0]] = qproj_rope(*ev)
                        qproj_matmuls(qp, wq_sb, 16, KT)
                        pending = (h, qp)

                    # finish the last projected head
                    ev = qproj_evict(pending[0], pending[1], nc.sync)
                    qrs[pending[0]] = qproj_rope(*ev)

                    # ---- tail: drain 5 attentions, covered by O(1)/O(0)
                    # partial accumulations ----
                    emit_attn(H - 5, qrs.pop(H - 5))
                    wo_sb1 = wqp.tile([P, KT, P], BF, tag="w")
                    nc.sync.dma_start(wo_sb1[:], wo4[1])
                    fp1 = psq.tile([P, NT], F32, tag="qp")
                    for k in range(H - 5):
                        nc.tensor.matmul(fp1[:], wo_sb1[:, k, :],
                                         attn_all[:, k, :],
                                         start=(k == 0), stop=False)
                    emit_attn(H - 4, qrs.pop(H - 4))
                    wo_sb0 = wqp.tile([P, KT, P], BF, tag="w")
                    nc.sync.dma_start(wo_sb0[:], wo4[0])
                    fp0 = psq.tile([P, NT], F32, tag="qp")
                    for k in range(H - 5):
                        nc.tensor.matmul(fp0[:], wo_sb0[:, k, :],
                                         attn_all[:, k, :],
                                         start=(k == 0), stop=False)
                    emit_attn(H - 3, qrs.pop(H - 3))
                    nc.tensor.matmul(fp1[:], wo_sb1[:, H - 5, :],
                                     attn_all[:, H - 5, :],
                                     start=False, stop=False)
                    nc.tensor.matmul(fp0[:], wo_sb0[:, H - 5, :],
                                     attn_all[:, H - 5, :],
                                     start=False, stop=False)
                    emit_attn(H - 2, qrs.pop(H - 2))
                    nc.tensor.matmul(fp1[:], wo_sb1[:, H - 4, :],
                                     attn_all[:, H - 4, :],
                                     start=False, stop=False)
                    nc.tensor.matmul(fp0[:], wo_sb0[:, H - 4, :],
                                     attn_all[:, H - 4, :],
                                     start=False, stop=False)
                    emit_attn(H - 1, qrs.pop(H - 1))
                    for k in range(H - 3, KT - 1):
                        nc.tensor.matmul(fp1[:], wo_sb1[:, k, :],
                                         attn_all[:, k, :],
                                         start=False, stop=False)
                        nc.tensor.matmul(fp0[:], wo_sb0[:, k, :],
                                         attn_all[:, k, :],
                                         start=False, stop=False)
                    nc.tensor.matmul(fp1[:], wo_sb1[:, KT - 1, :],
                                     attn_all[:, KT - 1, :],
                                     start=False, stop=True)
                    ob1 = outsb.tile([P, NT], BF, tag="ob")
                    nc.scalar.activation(ob1[:], fp1[:], AF.Identity,
                                         bias=bo_sb[:, 1:2])
                    nc.sync.dma_start(outp[1], ob1[:])
                    nc.tensor.matmul(fp0[:], wo_sb0[:, KT - 1, :],
                                     attn_all[:, KT - 1, :],
                                     start=False, stop=True)
                    ob0 = outsb.tile([P, NT], BF, tag="ob")
                    nc.scalar.activation(ob0[:], fp0[:], AF.Identity,
                                         bias=bo_sb[:, 0:1])
                    nc.sync.dma_start(outp[0], ob0[:])

                    # ---- output projection (rest) ----
                    for m in range(2, KT):
                        wo_sb = wqp.tile([P, KT, P], BF, tag="w")
                        nc.sync.dma_start(wo_sb[:], wo4[m])
                        fp = psq.tile([P, NT], F32, tag="qp")
                        for k in range(KT):
                            nc.tensor.matmul(fp[:], wo_sb[:, k, :],
                                             attn_all[:, k, :],
                                             start=(k == 0), stop=(k == KT - 1))
                        ob = outsb.tile([P, NT], BF, tag="ob")
                        nc.scalar.activation(ob[:], fp[:], AF.Identity,
                                             bias=bo_sb[:, m:m + 1])
                        nc.sync.dma_start(outp[m], ob[:])

    nc.compile()
    return nc


def _get_prog():
    global _PROG
    if _PROG is None:
        _PROG = _build()
    return _PROG


def _qcols(hf):
    return np.concatenate([np.arange(b * P, (b + 1) * P) for b in BLOCKS[hf]])


def _prepare_inmaps(inputs):
    pos = np.asarray(inputs["position_ids"])
    hs = np.asarray(inputs["hidden_states"], np.float32)
    Wq = np.asarray(inputs["Wq"], np.float32)
    bq = np.asarray(inputs["bq"], np.float32)
    Wkv = np.asarray(inputs["Wkv"], np.float32)
    bkv = np.asarray(inputs["bkv"], np.float32)
    Wo = np.asarray(inputs["Wo"], np.float32)
    bo = np.asarray(inputs["bo"], np.float32)

    shared = {
        "wq4": np.ascontiguousarray(
            Wq.reshape(H, P, KT, P).transpose(0, 3, 2, 1)).astype(bf16),
        "wo4": np.ascontiguousarray(
            Wo.reshape(KT, P, KT, P).transpose(0, 3, 2, 1)).astype(bf16),
        "wkv3": np.ascontiguousarray(
            Wkv.T.reshape(KT, P, 2 * D).transpose(1, 0, 2)).astype(bf16),
        "bq2": np.ascontiguousarray(bq.reshape(H, P).T),
        "bo2": np.ascontiguousarray(bo.reshape(KT, P).T),
        "bkv2": np.ascontiguousarray(bkv.reshape(2, P).T),
        "ident": np.eye(P, dtype=np.float32).astype(bf16),
    }

    invf = (1.0 / (ROPE_BASE ** (np.arange(0, ROT, 2, dtype=np.float32)
                                 / np.float32(ROT)))).astype(np.float32)
    in_maps = []
    for c in range(NCORES):
        b, hf = c // 2, c % 2
        qc = _qcols(hf)
        posb = pos[b].astype(np.float32)
        ang = invf[:, None] * posb[None, :]          # [32, S]
        cos1 = np.cos(ang).astype(np.float32)
        sin1 = np.sin(ang).astype(np.float32)
        cos2k = np.concatenate([cos1, cos1], 0)      # [64, S]
        sin2k = np.concatenate([sin1, sin1], 0)
        hidT = np.ascontiguousarray(hs[b].T)         # [HID, S]
        kpos = (np.arange(NKB)[None, :, None] * P
                + np.arange(P)[:, None, None])       # [P, NKB, 1]
        # Causal mask is over sequence INDICES (jnp.tril in the reference),
        # not position values; qc are the packed columns' sequence indices.
        mask = np.where(kpos <= qc[None, None, :], 0.0, -1e30).astype(np.float32)
        for t in range(NKB):
            lo, hi = MWIN[t]
            assert not mask[:, t, :lo].any() and not mask[:, t, hi:NVMAX[t]].any(), \
                f"mask outside window at t={t}"
        maskc = np.stack([mask[:, t, MWIN[t][0]:MWIN[t][1]]
                          for t in range(NKB)], axis=1)   # [P, NKB, 128]
        m = dict(shared)
        m["hid3"] = np.ascontiguousarray(
            hidT.reshape(KT, P, S).transpose(1, 0, 2)).astype(bf16)
        m["hidq"] = np.ascontiguousarray(
            hidT[:, qc].reshape(KT, P, NT).transpose(1, 0, 2)).astype(bf16)
        m["cosq"] = np.ascontiguousarray(cos2k[:, qc]).astype(bf16)
        m["sinq"] = np.ascontiguousarray(sin2k[:, qc]).astype(bf16)
        m["cosk"] = cos2k.astype(bf16)
        m["sink"] = sin2k.astype(bf16)
        m["maskt"] = np.ascontiguousarray(maskc).astype(bf16)
        in_maps.append(m)
    return in_maps


def _assemble(results):
    out = np.empty((B, S, HID), np.float32)
    for c in range(NCORES):
        b, hf = c // 2, c % 2
        outT = np.asarray(results[c]["out"], np.float32).reshape(HID, NT)
        out[b, _qcols(hf), :] = outT.T
    return out


def _run(inputs, trace=False, **kw):
    nc = _get_prog()
    in_maps = _prepare_inmaps(inputs)
    try:
        res = run_bass_kernel_spmd(nc, in_maps, list(range(NCORES)),
                                   trace=trace, **kw)
    except Exception:
        # transient device wedge (e.g. NRT_EXEC_UNIT_UNRECOVERABLE) — retry once
        res = run_bass_kernel_spmd(nc, in_maps, list(range(NCORES)),
                                   trace=trace, **kw)
    return _assemble(res.results), res


def kernel(**inputs):
    out, _ = _run(inputs)
    return out


# revision 12
# speedup vs baseline: 1.0107x; 1.0107x over previous
"""Trainium2 Bass kernel for the nn_Attention problem (B=4, S=1024, H=32, D=128).

Sharding: zero-collective data-parallel split. Each of the 8 cores owns one
(batch, half) pair: batch b = core//2, half = core%2. A half owns 4 of the 8
query blocks of 128 tokens, interleaved for causal balance:
  half 0 -> blocks [7, 5, 2, 0]   half 1 -> blocks [6, 4, 3, 1]
(both sum to 18 causal block-units, and the per-key-block column prefixes of
the two halves differ by at most one block, which makes the shared-program
NVMAX prefix sum optimal: 20 block-units vs 22 for the pairwise split).
Query columns are packed in DESCENDING block order so that key-block t only
needs a PREFIX of the packed columns. Each core computes Q proj (its tokens,
all heads), K/V proj (its whole batch), causal attention and the full output
projection for its tokens, then the host scatters the 8 token-slices back
into the full [B, S, HID] output.

Schedule (v4):
- Front is DMA-bandwidth-bound (~250 GB/s effective), so Q-proj inputs
  (hidq, wq0-3) are DMA'd first and heads 0-4 are projected while the KV
  inputs (wkv + 8 MB hid3) stream in; the KV loop then runs without stalls.
- Steady state emits, per iteration: attention for head h-5 in two 4-block
  phases with head-h Q-proj matmuls interleaved as PE filler, so the
  in-order engine streams never block the PE behind the softmax chain.
- The Q eviction + RoPE of head h are split across the iteration boundary:
  evict+swap-DMA at the start of iteration h+1 (Sync deps already met) and
  the RoPE vector ops LATE in iteration h+1, so the partition-swap DMA
  round-trip never stalls the DVE stream that feeds sums/exacc.
- Tail: the 5 drain heads' softmax chains are covered by partial O-proj
  accumulations for m=1,0 (two PSUM banks).

On-chip layout is transposed ([feature, token]) so every matmul has the
contraction dim on partitions with no transposes in the hot path.
"""

import numpy as np
import ml_dtypes

import concourse.bass as bass
import concourse.tile as tile
from concourse import bacc, mybir
from concourse.bass_utils import run_bass_kernel_spmd

B, S, H, D = 4, 1024, 32, 128
HID = H * D          # 4096
ROT = D // 2         # 64
HR = ROT // 2        # 32
ROPE_BASE = 10000.0
P = 128
NT = 512             # query tokens per core
NCORES = 8
KT = HID // P        # 32 contraction tiles
NKB = S // P         # 8 key blocks
SCALE = float(D) ** -0.5
PRE = 5              # heads projected before the KV phase (pipeline depth)

BLOCKS = [[7, 5, 2, 0], [6, 4, 3, 1]]
NVMAX = [512, 512, 384, 384, 256, 256, 128, 128]
MWIN = [(384, 512), (384, 512), (256, 384), (256, 384),
        (128, 256), (128, 256), (0, 128), (0, 128)]

BF = mybir.dt.bfloat16
F32 = mybir.dt.float32
AF = mybir.ActivationFunctionType
bf16 = ml_dtypes.bfloat16

_PROG = None


def _build():
    nc = bacc.Bacc("TRN2", target_bir_lowering=False, debug=False,
                   num_devices=NCORES)
    dp = nc.declare_dram_parameter
    hid3 = dp("hid3", [P, KT, S], BF, False)        # [p, k, t] = hidden[b,t,k*128+p]
    hidq = dp("hidq", [P, KT, NT], BF, False)       # packed query columns
    wq4 = dp("wq4", [H, P, KT, P], BF, False)       # [h,p,k,d] = Wq[h*128+d, k*128+p]
    wo4 = dp("wo4", [KT, P, KT, P], BF, False)      # [m,p,k,d] = Wo[m*128+d, k*128+p]
    wkv3 = dp("wkv3", [P, KT, 2 * D], BF, False)    # [p,k,c] = Wkv[c, k*128+p]
    bq2 = dp("bq2", [P, H], F32, False)
    bo2 = dp("bo2", [P, KT], F32, False)
    bkv2 = dp("bkv2", [P, 2], F32, False)
    cosq = dp("cosq", [ROT, NT], BF, False)
    sinq = dp("sinq", [ROT, NT], BF, False)
    cosk = dp("cosk", [ROT, S], BF, False)
    sink = dp("sink", [ROT, S], BF, False)
    maskt = dp("maskt", [P, NKB, P], BF, False)     # additive 0/-1e30, window only
    ident = dp("ident", [P, P], BF, False)
    outp = dp("out", [KT, P, NT], BF, True)        # [m, dd, c] = out.T slice

    with tile.TileContext(nc) as tc:
        with (
            tc.tile_pool(name="const", bufs=1) as constp,
            tc.tile_pool(name="persist", bufs=1) as persist,
            tc.tile_pool(name="wqp", bufs=4) as wqp,
            tc.tile_pool(name="qtmp", bufs=3) as qtmp,
            tc.tile_pool(name="qrp", bufs=6) as qrp,
            tc.tile_pool(name="expp", bufs=5) as expp,
            tc.tile_pool(name="exap", bufs=2) as exap,
            tc.tile_pool(name="nrm", bufs=2) as nrm,
            tc.tile_pool(name="outsb", bufs=2) as outsb,
        ):
            attn_all = persist.tile([P, KT, NT], BF, tag="attn")
            kbf = persist.tile([P, S], BF, tag="kbf")
            vnat = persist.tile([P, NKB, P], BF, tag="vnat")
            hidq_sb = persist.tile([P, KT, NT], BF, tag="hidq")

            # ---- earliest DMAs, interleaved so qproj(0) streams: the PE can
            # start on (hidq c1, wq0 c1) while the rest arrives ----
            nc.sync.dma_start(hidq_sb[:, 0:8, :], hidq[:, 0:8, :])
            wq_pre = {}
            w = wqp.tile([P, KT, P], BF, tag="w")
            nc.sync.dma_start(w[:, 0:8, :], wq4[0, :, 0:8, :])
            nc.sync.dma_start(w[:, 8:32, :], wq4[0, :, 8:32, :])
            wq_pre[0] = w
            for h in range(1, 4):
                nc.sync.dma_start(hidq_sb[:, 8 * h:8 * h + 8, :],
                                  hidq[:, 8 * h:8 * h + 8, :])
                w = wqp.tile([P, KT, P], BF, tag="w")
                nc.sync.dma_start(w[:], wq4[h])
                wq_pre[h] = w
            bq_sb = constp.tile([P, H], F32, tag="bq")
            nc.sync.dma_start(bq_sb[:], bq2[:])
            cosq_sb = constp.tile([ROT, NT], BF, tag="cq")
            nc.sync.dma_start(cosq_sb[:], cosq[:])
            sinq_sb = constp.tile([ROT, NT], BF, tag="sq")
            nc.sync.dma_start(sinq_sb[:], sinq[:])

            with tc.tile_pool(name="psq", bufs=2, space="PSUM") as psq:

                def qproj_matmuls(qp, wq_sb, k0, k1):
                    for k in range(k0, k1):
                        nc.tensor.matmul(qp[:], wq_sb[:, k, :], hidq_sb[:, k, :],
                                         start=(k == 0), stop=(k == KT - 1))

                def qproj_evict(h, qp, swap_engine):
                    qfp = qtmp.tile([P, NT], BF, tag="qfp")
                    nc.scalar.activation(qfp[0:ROT, :], qp[0:ROT, :], AF.Identity,
                                         bias=bq_sb[0:ROT, h:h + 1])
                    qsw = qtmp.tile([ROT, NT], BF, tag="qsw")
                    swap_engine.dma_start(qsw[0:HR, :], qfp[HR:ROT, :])
                    swap_engine.dma_start(qsw[HR:ROT, :], qfp[0:HR, :])
                    nc.scalar.activation(qfp[ROT:P, :], qp[ROT:P, :], AF.Identity,
                                         bias=bq_sb[ROT:P, h:h + 1])
                    return qfp, qsw

                def qproj_rope(qfp, qsw):
                    m1q = qtmp.tile([ROT, NT], BF, tag="m1q")
                    m2q = qtmp.tile([ROT, NT], BF, tag="m2q")
                    qr = qrp.tile([P, NT], BF, tag="qr")
                    nc.vector.tensor_mul(m1q[:], qfp[0:ROT, :], cosq_sb[:])
                    nc.vector.tensor_mul(m2q[:], qsw[:], sinq_sb[:])
                    nc.vector.tensor_sub(qr[0:HR, :], m1q[0:HR, :], m2q[0:HR, :])
                    nc.vector.tensor_add(qr[HR:ROT, :], m1q[HR:ROT, :],
                                         m2q[HR:ROT, :])
                    nc.vector.tensor_copy(qr[ROT:P, :], qfp[ROT:P, :])
                    return qr

                def emit_qproj_front(h, wq_sb):
                    qp = psq.tile([P, NT], F32, tag="qp")
                    qproj_matmuls(qp, wq_sb, 0, KT)
                    # front: swap via ScalarE so the Sync bulk stream is
                    # never head-of-line blocked waiting on an eviction
                    qfp, qsw = qproj_evict(h, qp, nc.scalar)
                    return qproj_rope(qfp, qsw)

                qrs = {}
                qrs[0] = emit_qproj_front(0, wq_pre[0])
                qrs[1] = emit_qproj_front(1, wq_pre[1])
                qrs[2] = emit_qproj_front(2, wq_pre[2])

                with (
                    tc.tile_pool(name="hidp", bufs=8) as hidp,
                    tc.tile_pool(name="wkvp", bufs=1) as wkvp,
                    tc.tile_pool(name="kvtmp", bufs=1) as kvtmp,
                    tc.tile_pool(name="pskv", bufs=1, space="PSUM") as pskv,
                    tc.tile_pool(name="pstr", bufs=2, space="PSUM") as pstr,
                ):
                    kps = [pskv.tile([P, S // 2], F32, name=f"kps{j}",
                                     tag=f"kps{j}") for j in range(2)]
                    vps = [pskv.tile([P, S // 2], F32, name=f"vps{j}",
                                     tag=f"vps{j}") for j in range(2)]
                    wkv_all = wkvp.tile([P, KT, 2 * D], BF, tag="wkvall")
                    nc.sync.dma_start(wkv_all[:, 0:8, :], wkv3[:, 0:8, :])
                    nc.sync.dma_start(wkv_all[:, 8:20, :], wkv3[:, 8:20, :])
                    nc.sync.dma_start(wkv_all[:, 20:32, :], wkv3[:, 20:32, :])

                    hh0 = hidp.tile([P, 1, S], BF, name="hh0", tag="hh")
                    nc.sync.dma_start(hh0[:], hid3[:, 0:1, :])
                    hhs = []
                    for g in range(15):
                        hh = hidp.tile([P, 2, S], BF, tag="hh")
                        nc.sync.dma_start(hh[:],
                                          hid3[:, 1 + 2 * g:3 + 2 * g, :])
                        hhs.append(hh)
                        if g == 5:
                            # first 8 hh tiles (7 issues) fit the pool without
                            # buffer waits; slot wq4/wq5 in before the paced rest
                            w = wqp.tile([P, KT, P], BF, tag="w")
                            nc.sync.dma_start(w[:], wq4[4])
                            wq_pre[4] = w
                            w = wqp.tile([P, KT, P], BF, tag="w")
                            nc.sync.dma_start(w[:], wq4[5])
                            wq_pre[5] = w
                    hh31 = hidp.tile([P, 1, S], BF, name="hh31", tag="hh")
                    nc.sync.dma_start(hh31[:], hid3[:, 31:32, :])

                    def hh_slice(k):
                        if k == 0:
                            return hh0[:, 0, :]
                        if k == KT - 1:
                            return hh31[:, 0, :]
                        return hhs[(k - 1) // 2][:, (k - 1) % 2, :]

                    cosk_sb = constp.tile([ROT, S], BF, tag="ck")
                    nc.sync.dma_start(cosk_sb[:], cosk[:])
                    sink_sb = constp.tile([ROT, S], BF, tag="sk")
                    nc.sync.dma_start(sink_sb[:], sink[:])
                    bkv_sb = constp.tile([P, 2], F32, tag="bkv")
                    nc.sync.dma_start(bkv_sb[:], bkv2[:])
                    id_sb = constp.tile([P, P], BF, tag="id")
                    nc.sync.dma_start(id_sb[:], ident[:])
                    mask_sb = constp.tile([P, NKB, P], BF, tag="mask")
                    nc.sync.dma_start(mask_sb[:], maskt[:])
                    bo_sb = constp.tile([P, KT], F32, tag="bo")
                    nc.sync.dma_start(bo_sb[:], bo2[:])
                    w = wqp.tile([P, KT, P], BF, tag="w")
                    nc.sync.dma_start(w[:], wq4[6])
                    wq_pre[6] = w
                    w = wqp.tile([P, KT, P], BF, tag="w")
                    nc.sync.dma_start(w[:], wq4[7])
                    wq_pre[7] = w
                    ones_sb = constp.tile([P, 1], BF, tag="ones")
                    nc.gpsimd.memset(ones_sb[:], 1.0)

                    qrs[3] = emit_qproj_front(3, wq_pre.pop(3))
                    qrs[4] = emit_qproj_front(4, wq_pre.pop(4))

                    # KV matmuls (kps pair shares a weight load, then vps pair)
                    for k in range(KT):
                        st, sp_ = (k == 0), (k == KT - 1)
                        hhk = hh_slice(k)
                        for j in range(2):
                            nc.tensor.matmul(kps[j][:], wkv_all[:, k, 0:D],
                                             hhk[:, j * 512:(j + 1) * 512],
                                             start=st, stop=sp_)
                        for j in range(2):
                            nc.tensor.matmul(vps[j][:], wkv_all[:, k, D:2 * D],
                                             hhk[:, j * 512:(j + 1) * 512],
                                             start=st, stop=sp_)

                    # ---- KV epilogue: evictions, K RoPE, V PE-transpose ----
                    kfp = kvtmp.tile([P, S], BF, tag="kfp")
                    vbf = kvtmp.tile([P, S], BF, tag="vbf")
                    for j in range(2):
                        sl = slice(j * 512, (j + 1) * 512)
                        nc.scalar.activation(kfp[:, sl], kps[j][:], AF.Identity,
                                             bias=bkv_sb[:, 0:1])
                        nc.scalar.activation(vbf[:, sl], vps[j][:], AF.Identity,
                                             bias=bkv_sb[:, 1:2])
                    ksw = kvtmp.tile([ROT, S], BF, tag="ksw")
                    nc.scalar.dma_start(ksw[0:HR, :], kfp[HR:ROT, :])
                    nc.scalar.dma_start(ksw[HR:ROT, :], kfp[0:HR, :])
                    m1 = kvtmp.tile([ROT, S], BF, tag="m1")
                    m2 = kvtmp.tile([ROT, S], BF, tag="m2")
                    nc.vector.tensor_mul(m1[:], kfp[0:ROT, :], cosk_sb[:])
                    nc.vector.tensor_mul(m2[:], ksw[:], sink_sb[:])
                    nc.vector.tensor_sub(kbf[0:HR, :], m1[0:HR, :], m2[0:HR, :])
                    nc.vector.tensor_add(kbf[HR:ROT, :], m1[HR:ROT, :],
                                         m2[HR:ROT, :])
                    nc.vector.tensor_copy(kbf[ROT:P, :], kfp[ROT:P, :])
                    for t in range(NKB):
                        vt = pstr.tile([P, P], BF, tag="vt")
                        nc.tensor.transpose(vt[:], vbf[:, t * P:(t + 1) * P],
                                            id_sb[:])
                        nc.vector.tensor_copy(vnat[:, t, :], vt[:])

                with (
                    tc.tile_pool(name="pss", bufs=4, space="PSUM") as pss,
                    tc.tile_pool(name="pso", bufs=1, space="PSUM") as pso,
                    tc.tile_pool(name="pssum", bufs=1, space="PSUM") as pssum,
                ):
                    def attn_scores(qr, trange):
                        sps = []
                        for t in trange:
                            nv = NVMAX[t]
                            sp = pss.tile([P, NT], F32, tag="sp")
                            nc.tensor.matmul(sp[:, 0:nv],
                                             kbf[:, t * P:(t + 1) * P],
                                             qr[:, 0:nv], start=True, stop=True)
                            sps.append((t, sp))
                        for t, sp in sps:
                            lo, hi = MWIN[t]
                            nc.vector.tensor_add(sp[:, lo:hi], sp[:, lo:hi],
                                                 mask_sb[:, t, :])
                        return sps

                    def attn_pv(sps, op):
                        exs = []
                        for t, sp in sps:
                            ex = expp.tile([P, NT], BF, tag="ex")
                            nc.scalar.activation(ex[:, 0:NVMAX[t]],
                                                 sp[:, 0:NVMAX[t]], AF.Exp,
                                                 scale=SCALE)
                            exs.append((t, ex))
                        for t, ex in exs:
                            nc.tensor.matmul(op[:, 0:NVMAX[t]], vnat[:, t, :],
                                             ex[:, 0:NVMAX[t]],
                                             start=(t == 0), stop=(t == NKB - 1))
                        return exs

                    def attn_exacc(exacc, exs):
                        for t, ex in exs:
                            if t == 0:
                                nc.vector.tensor_copy(exacc[:], ex[:])
                            else:
                                nc.vector.tensor_add(exacc[:, 0:NVMAX[t]],
                                                     exacc[:, 0:NVMAX[t]],
                                                     ex[:, 0:NVMAX[t]])

                    def attn_end(h, op, exacc):
                        sums = pssum.tile([1, NT], F32, tag="sums")
                        nc.tensor.matmul(sums[:], ones_sb[:], exacc[:],
                                         start=True, stop=True)
                        rs = nrm.tile([1, NT], F32, tag="rs")
                        nc.vector.reciprocal_approx_fast(rs[:], sums[:])
                        rb = nrm.tile([P, NT], F32, tag="rb")
                        nc.gpsimd.partition_broadcast(rb[:], rs[:])
                        nc.vector.tensor_mul(attn_all[:, h, :], op[:], rb[:])

                    def emit_attn(h, qr):
                        op = pso.tile([P, NT], F32, tag="op")
                        exacc = exap.tile([P, NT], BF, tag="exacc")
                        sps1 = attn_scores(qr, range(0, 4))
                        exs1 = attn_pv(sps1, op)
                        attn_exacc(exacc, exs1)
                        sps2 = attn_scores(qr, range(4, NKB))
                        exs2 = attn_pv(sps2, op)
                        attn_exacc(exacc, exs2)
                        attn_end(h, op, exacc)

                    # ---- steady-state head pipeline (depth PRE=5) ----
                    pending = None   # (h, qp) awaiting evict+rope
                    for h in range(PRE, H):
                        if h in wq_pre:
                            wq_sb = wq_pre.pop(h)
                        else:
                            wq_sb = wqp.tile([P, KT, P], BF, tag="w")
                            nc.sync.dma_start(wq_sb[:], wq4[h])
                        qp = psq.tile([P, NT], F32, tag="qp")
                        ha = h - PRE
                        qr_a = qrs.pop(ha)
                        op = pso.tile([P, NT], F32, tag="op")
                        exacc = exap.tile([P, NT], BF, tag="exacc")
                        if pending is not None:
                            hp, qpp = pending
                            ev = qproj_evict(hp, qpp, nc.sync)
                        sps1 = attn_scores(qr_a, range(0, 4))
                        qproj_matmuls(qp, wq_sb, 0, 6)
                        exs1 = attn_pv(sps1, op)
                        qproj_matmuls(qp, wq_sb, 6, 12)
                        attn_exacc(exacc, exs1)
                        sps2 = attn_scores(qr_a, range(4, NKB))
                        exs2 = attn_pv(sps2, op)
                        qproj_matmuls(qp, wq_sb, 12, 16)
                        attn_exacc(exacc, exs2)
                        attn_end(ha, op, exacc)
                        if pending is not None:
                            qrs[pending[0]] = qproj_rope(*ev)
                        qproj_matmuls(qp, wq_sb, 16, KT)
                        pending = (h, qp)

                    # finish the last projected head
                    ev = qproj_evict(pending[0], pending[1], nc.sync)
                    qrs[pending[0]] = qproj_rope(*ev)

                    # ---- tail: drain 5 attentions, covered by O(1)/O(0)
                    # partial accumulations ----
                    emit_attn(H - 5, qrs.pop(H - 5))
                    wo_sb1 = wqp.tile([P, KT, P], BF, tag="w")
                    nc.sync.dma_start(wo_sb1[:], wo4[1])
                    fp1 = psq.tile([P, NT], F32, tag="qp")
                    for k in range(H - 5):
                        nc.tensor.matmul(fp1[:], wo_sb1[:, k, :],
                                         attn_all[:, k, :],
                                         start=(k == 0), stop=False)
                    emit_attn(H - 4, qrs.pop(H - 4))
                    wo_sb0 = wqp.tile([P, KT, P], BF, tag="w")
                    nc.sync.dma_start(wo_sb0[:], wo4[0])
                    fp0 = psq.tile([P, NT], F32, tag="qp")
                    for k in range(H - 5):
                        nc.tensor.matmul(fp0[:], wo_sb0[:, k, :],
                                         attn_all[:, k, :],
                                         start=(k == 0), stop=False)
                    emit_attn(H - 3, qrs.pop(H - 3))
                    nc.tensor.matmul(fp1[:], wo_sb1[:, H - 5, :],
                                     attn_all[:, H - 5, :],
                                     start=False, stop=False)
                    nc.tensor.matmul(fp0[:], wo_sb0[:, H - 5, :],
                                     attn_all[:, H - 5, :],
                                     start=False, stop=False)
                    emit_attn(H - 2, qrs.pop(H - 2))
                    nc.tensor.matmul(fp1[:], wo_sb1[:, H - 4, :],
                                     attn_all[:, H - 4, :],
                                     start=False, stop=False)
                    nc.tensor.matmul(fp0[:], wo_sb0[:, H - 4, :],
                                     attn_all[:, H - 4, :],
                                     start=False, stop=False)
                    emit_attn(H - 1, qrs.pop(H - 1))
                    for k in range(H - 3, KT - 1):
                        nc.tensor.matmul(fp1[:], wo_sb1[:, k, :],
                                         attn_all[:, k, :],
                                         start=False, stop=False)
                        nc.tensor.matmul(fp0[:], wo_sb0[:, k, :],
                                         attn_all[:, k, :],
                                         start=False, stop=False)
                    nc.tensor.matmul(fp1[:], wo_sb1[:, KT - 1, :],
                                     attn_all[:, KT - 1, :],
                                     start=False, stop=True)
                    ob1 = outsb.tile([P, NT], BF, tag="ob")
                    nc.scalar.activation(ob1[:], fp1[:], AF.Identity,
                                         bias=bo_sb[:, 1:2])
                    nc.sync.dma_start(outp[1], ob1[:])
                    nc.tensor.matmul(fp0[:], wo_sb0[:, KT - 1, :],
                                     attn_all[:, KT - 1, :],
                                     start=False, stop=True)
                    ob0 = outsb.tile([P, NT], BF, tag="ob")
                    nc.scalar.activation(ob0[:], fp0[:], AF.Identity,
                                         bias=bo_sb[:, 0:1])
                    nc.sync.dma_start(outp[0], ob0[:])

                    # ---- output projection (rest) ----
                    for m in range(2, KT):
                        wo_sb = wqp.tile([P, KT, P], BF, tag="w")
                        nc.sync.dma_start(wo_sb[:], wo4[m])
                        fp = psq.tile([P, NT], F32, tag="qp")
                        for k in range(KT):
                            nc.tensor.matmul(fp[:], wo_sb[:, k, :],
                                             attn_all[:, k, :],
                                             start=(k == 0), stop=(k == KT - 1))
                        ob = outsb.tile([P, NT], BF, tag="ob")
                        nc.scalar.activation(ob[:], fp[:], AF.Identity,
                                             bias=bo_sb[:, m:m + 1])
                        nc.sync.dma_start(outp[m], ob[:])

    nc.compile()
    return nc


def _get_prog():
    global _PROG
    if _PROG is None:
        _PROG = _build()
    return _PROG


def _qcols(hf):
    return np.concatenate([np.arange(b * P, (b + 1) * P) for b in BLOCKS[hf]])


def _prepare_inmaps(inputs):
    pos = np.asarray(inputs["position_ids"])
    hs = np.asarray(inputs["hidden_states"], np.float32)
    Wq = np.asarray(inputs["Wq"], np.float32)
    bq = np.asarray(inputs["bq"], np.float32)
    Wkv = np.asarray(inputs["Wkv"], np.float32)
    bkv = np.asarray(inputs["bkv"], np.float32)
    Wo = np.asarray(inputs["Wo"], np.float32)
    bo = np.asarray(inputs["bo"], np.float32)

    shared = {
        "wq4": np.ascontiguousarray(
            Wq.reshape(H, P, KT, P).transpose(0, 3, 2, 1)).astype(bf16),
        "wo4": np.ascontiguousarray(
            Wo.reshape(KT, P, KT, P).transpose(0, 3, 2, 1)).astype(bf16),
        "wkv3": np.ascontiguousarray(
            Wkv.T.reshape(KT, P, 2 * D).transpose(1, 0, 2)).astype(bf16),
        "bq2": np.ascontiguousarray(bq.reshape(H, P).T),
        "bo2": np.ascontiguousarray(bo.reshape(KT, P).T),
        "bkv2": np.ascontiguousarray(bkv.reshape(2, P).T),
        "ident": np.eye(P, dtype=np.float32).astype(bf16),
    }

    invf = (1.0 / (ROPE_BASE ** (np.arange(0, ROT, 2, dtype=np.float32)
                                 / np.float32(ROT)))).astype(np.float32)
    in_maps = []
    for c in range(NCORES):
        b, hf = c // 2, c % 2
        qc = _qcols(hf)
        posb = pos[b].astype(np.float32)
        ang = invf[:, None] * posb[None, :]          # [32, S]
        cos1 = np.cos(ang).astype(np.float32)
        sin1 = np.sin(ang).astype(np.float32)
        cos2k = np.concatenate([cos1, cos1], 0)      # [64, S]
        sin2k = np.concatenate([sin1, sin1], 0)
        hidT = np.ascontiguousarray(hs[b].T)         # [HID, S]
        kpos = (np.arange(NKB)[None, :, None] * P
                + np.arange(P)[:, None, None])       # [P, NKB, 1]
        # Causal mask is over sequence INDICES (jnp.tril in the reference),
        # not position values; qc are the packed columns' sequence indices.
        mask = np.where(kpos <= qc[None, None, :], 0.0, -1e30).astype(np.float32)
        for t in range(NKB):
            lo, hi = MWIN[t]
            assert not mask[:, t, :lo].any() and not mask[:, t, hi:NVMAX[t]].any(), \
                f"mask outside window at t={t}"
        maskc = np.stack([mask[:, t, MWIN[t][0]:MWIN[t][1]]
                          for t in range(NKB)], axis=1)   # [P, NKB, 128]
        m = dict(shared)
        m["hid3"] = np.ascontiguousarray(
            hidT.reshape(KT, P, S).transpose(1, 0, 2)).astype(bf16)
        m["hidq"] = np.ascontiguousarray(
            hidT[:, qc].reshape(KT, P, NT).transpose(1, 0, 2)).astype(bf16)
        m["cosq"] = np.ascontiguousarray(cos2k[:, qc]).astype(bf16)
        m["sinq"] = np.ascontiguousarray(sin2k[:, qc]).astype(bf16)
        m["cosk"] = cos2k.astype(bf16)
        m["sink"] = sin2k.astype(bf16)
        m["maskt"] = np.ascontiguousarray(maskc).astype(bf16)
        in_maps.append(m)
    return in_maps


def _assemble(results):
    out = np.empty((B, S, HID), np.float32)
    for c in range(NCORES):
        b, hf = c // 2, c % 2
        outT = np.asarray(results[c]["out"], np.float32).reshape(HID, NT)
        out[b, _qcols(hf), :] = outT.T
    return out


def _run(inputs, trace=False, **kw):
    nc = _get_prog()
    in_maps = _prepare_inmaps(inputs)
    try:
        res = run_bass_kernel_spmd(nc, in_maps, list(range(NCORES)),
                                   trace=trace, **kw)
    except Exception:
        # transient device wedge (e.g. NRT_EXEC_UNIT_UNRECOVERABLE) — retry once
        res = run_bass_kernel_spmd(nc, in_maps, list(range(NCORES)),
                                   trace=trace, **kw)
    return _assemble(res.results), res


def kernel(**inputs):
    out, _ = _run(inputs)
    return out


# revision 14
# speedup vs baseline: 1.0173x; 1.0066x over previous
"""Trainium2 Bass kernel for the nn_Attention problem (B=4, S=1024, H=32, D=128).

Sharding: zero-collective data-parallel split. Each of the 8 cores owns one
(batch, half) pair: batch b = core//2, half = core%2. A half owns 4 of the 8
query blocks of 128 tokens, interleaved for causal balance:
  half 0 -> blocks [7, 5, 2, 0]   half 1 -> blocks [6, 4, 3, 1]
(both sum to 18 causal block-units, and the per-key-block column prefixes of
the two halves differ by at most one block, which makes the shared-program
NVMAX prefix sum optimal: 20 block-units vs 22 for the pairwise split).
Query columns are packed in DESCENDING block order so that key-block t only
needs a PREFIX of the packed columns. Each core computes Q proj (its tokens,
all heads), K/V proj (its whole batch), causal attention and the full output
projection for its tokens, then the host scatters the 8 token-slices back
into the full [B, S, HID] output.

Schedule (v4):
- Front is DMA-bandwidth-bound (~250 GB/s effective), so Q-proj inputs
  (hidq, wq0-3) are DMA'd first and heads 0-4 are projected while the KV
  inputs (wkv + 8 MB hid3) stream in; the KV loop then runs without stalls.
- Steady state emits, per iteration: attention for head h-5 in two 4-block
  phases with head-h Q-proj matmuls interleaved as PE filler, so the
  in-order engine streams never block the PE behind the softmax chain.
- The Q eviction + RoPE of head h are split across the iteration boundary:
  evict+swap-DMA at the start of iteration h+1 (Sync deps already met) and
  the RoPE vector ops LATE in iteration h+1, so the partition-swap DMA
  round-trip never stalls the DVE stream that feeds sums/exacc.
- Tail: the 5 drain heads' softmax chains are covered by partial O-proj
  accumulations for m=1,0 (two PSUM banks).

On-chip layout is transposed ([feature, token]) so every matmul has the
contraction dim on partitions with no transposes in the hot path.
"""

import numpy as np
import ml_dtypes

import concourse.bass as bass
import concourse.tile as tile
from concourse import bacc, mybir
from concourse.bass_utils import run_bass_kernel_spmd

B, S, H, D = 4, 1024, 32, 128
HID = H * D          # 4096
ROT = D // 2         # 64
HR = ROT // 2        # 32
ROPE_BASE = 10000.0
P = 128
NT = 512             # query tokens per core
NCORES = 8
KT = HID // P        # 32 contraction tiles
NKB = S // P         # 8 key blocks
SCALE = float(D) ** -0.5
PRE = 5              # heads projected before the KV phase (pipeline depth)

BLOCKS = [[7, 5, 2, 0], [6, 4, 3, 1]]
NVMAX = [512, 512, 384, 384, 256, 256, 128, 128]
MWIN = [(384, 512), (384, 512), (256, 384), (256, 384),
        (128, 256), (128, 256), (0, 128), (0, 128)]

BF = mybir.dt.bfloat16
F32 = mybir.dt.float32
AF = mybir.ActivationFunctionType
bf16 = ml_dtypes.bfloat16

_PROG = None


def _build():
    nc = bacc.Bacc("TRN2", target_bir_lowering=False, debug=False,
                   num_devices=NCORES)
    dp = nc.declare_dram_parameter
    hid3 = dp("hid3", [P, KT, S], BF, False)        # [p, k, t] = hidden[b,t,k*128+p]
    hidq = dp("hidq", [P, KT, NT], BF, False)       # packed query columns
    wq4 = dp("wq4", [H, P, KT, P], BF, False)       # [h,p,k,d] = Wq[h*128+d, k*128+p]
    wo4 = dp("wo4", [KT, P, KT, P], BF, False)      # [m,p,k,d] = Wo[m*128+d, k*128+p]
    wkv3 = dp("wkv3", [P, KT, 2 * D], BF, False)    # [p,k,c] = Wkv[c, k*128+p]
    bq2 = dp("bq2", [P, H], F32, False)
    bo2 = dp("bo2", [P, KT], F32, False)
    bkv2 = dp("bkv2", [P, 2], F32, False)
    cosq = dp("cosq", [ROT, NT], BF, False)
    sinq = dp("sinq", [ROT, NT], BF, False)
    cosk = dp("cosk", [ROT, S], BF, False)
    sink = dp("sink", [ROT, S], BF, False)
    maskt = dp("maskt", [P, NKB, P], BF, False)     # additive 0/-1e30, window only
    ident = dp("ident", [P, P], BF, False)
    outp = dp("out", [KT, P, NT], BF, True)        # [m, dd, c] = out.T slice

    with tile.TileContext(nc) as tc:
        with (
            tc.tile_pool(name="const", bufs=1) as constp,
            tc.tile_pool(name="persist", bufs=1) as persist,
            tc.tile_pool(name="wqp", bufs=4) as wqp,
            tc.tile_pool(name="qtmp", bufs=3) as qtmp,
            tc.tile_pool(name="qrp", bufs=6) as qrp,
            tc.tile_pool(name="expp", bufs=5) as expp,
            tc.tile_pool(name="exap", bufs=2) as exap,
            tc.tile_pool(name="nrm", bufs=2) as nrm,
            tc.tile_pool(name="outsb", bufs=2) as outsb,
        ):
            attn_all = persist.tile([P, KT, NT], BF, tag="attn")
            kbf = persist.tile([P, S], BF, tag="kbf")
            vnat = persist.tile([P, NKB, P], BF, tag="vnat")
            hidq_sb = persist.tile([P, KT, NT], BF, tag="hidq")

            # ---- earliest DMAs, interleaved so qproj(0) streams: the PE can
            # start on (hidq c1, wq0 c1) while the rest arrives ----
            nc.sync.dma_start(hidq_sb[:, 0:8, :], hidq[:, 0:8, :])
            wq_pre = {}
            w = wqp.tile([P, KT, P], BF, tag="w")
            nc.sync.dma_start(w[:, 0:8, :], wq4[0, :, 0:8, :])
            nc.sync.dma_start(w[:, 8:32, :], wq4[0, :, 8:32, :])
            wq_pre[0] = w
            for h in range(1, 4):
                nc.sync.dma_start(hidq_sb[:, 8 * h:8 * h + 8, :],
                                  hidq[:, 8 * h:8 * h + 8, :])
                w = wqp.tile([P, KT, P], BF, tag="w")
                nc.sync.dma_start(w[:], wq4[h])
                wq_pre[h] = w
            bq_sb = constp.tile([P, H], F32, tag="bq")
            nc.sync.dma_start(bq_sb[:], bq2[:])
            cosq_sb = constp.tile([ROT, NT], BF, tag="cq")
            nc.sync.dma_start(cosq_sb[:], cosq[:])
            sinq_sb = constp.tile([ROT, NT], BF, tag="sq")
            nc.sync.dma_start(sinq_sb[:], sinq[:])
            # warm up the ScalarE activation table (the lazy 1.3us
            # ACT_TABLE_LOAD otherwise lands on the first eviction)
            scr = constp.tile([1, 1], F32, tag="scr")
            nc.gpsimd.memset(scr[:], 0.0)
            nc.scalar.activation(scr[:], scr[:], AF.Identity)
            # ones row for PE-side reciprocal broadcast (tail heads)
            ones_row = constp.tile([1, P], BF, tag="onesr")
            nc.gpsimd.memset(ones_row[:], 1.0)

            with tc.tile_pool(name="psq", bufs=2, space="PSUM") as psq:

                def qproj_matmuls(qp, wq_sb, k0, k1):
                    for k in range(k0, k1):
                        nc.tensor.matmul(qp[:], wq_sb[:, k, :], hidq_sb[:, k, :],
                                         start=(k == 0), stop=(k == KT - 1))

                def qproj_evict(h, qp, swap_engine):
                    qfp = qtmp.tile([P, NT], BF, tag="qfp")
                    nc.scalar.activation(qfp[0:ROT, :], qp[0:ROT, :], AF.Identity,
                                         bias=bq_sb[0:ROT, h:h + 1])
                    qsw = qtmp.tile([ROT, NT], BF, tag="qsw")
                    swap_engine.dma_start(qsw[0:HR, :], qfp[HR:ROT, :])
                    swap_engine.dma_start(qsw[HR:ROT, :], qfp[0:HR, :])
                    nc.scalar.activation(qfp[ROT:P, :], qp[ROT:P, :], AF.Identity,
                                         bias=bq_sb[ROT:P, h:h + 1])
                    return qfp, qsw

                def qproj_rope(qfp, qsw):
                    m1q = qtmp.tile([ROT, NT], BF, tag="m1q")
                    m2q = qtmp.tile([ROT, NT], BF, tag="m2q")
                    qr = qrp.tile([P, NT], BF, tag="qr")
                    nc.vector.tensor_mul(m1q[:], qfp[0:ROT, :], cosq_sb[:])
                    nc.vector.tensor_mul(m2q[:], qsw[:], sinq_sb[:])
                    nc.vector.tensor_sub(qr[0:HR, :], m1q[0:HR, :], m2q[0:HR, :])
                    nc.vector.tensor_add(qr[HR:ROT, :], m1q[HR:ROT, :],
                                         m2q[HR:ROT, :])
                    nc.vector.tensor_copy(qr[ROT:P, :], qfp[ROT:P, :])
                    return qr

                def emit_qproj_front(h, wq_sb):
                    qp = psq.tile([P, NT], F32, tag="qp")
                    qproj_matmuls(qp, wq_sb, 0, KT)
                    # front: swap via ScalarE so the Sync bulk stream is
                    # never head-of-line blocked waiting on an eviction
                    qfp, qsw = qproj_evict(h, qp, nc.scalar)
                    return qproj_rope(qfp, qsw)

                qrs = {}
                qrs[0] = emit_qproj_front(0, wq_pre[0])
                qrs[1] = emit_qproj_front(1, wq_pre[1])
                qrs[2] = emit_qproj_front(2, wq_pre[2])

                with (
                    tc.tile_pool(name="hidp", bufs=8) as hidp,
                    tc.tile_pool(name="wkvp", bufs=1) as wkvp,
                    tc.tile_pool(name="kvtmp", bufs=1) as kvtmp,
                    tc.tile_pool(name="pskv", bufs=1, space="PSUM") as pskv,
                    tc.tile_pool(name="pstr", bufs=2, space="PSUM") as pstr,
                ):
                    kps = [pskv.tile([P, S // 2], F32, name=f"kps{j}",
                                     tag=f"kps{j}") for j in range(2)]
                    vps = [pskv.tile([P, S // 2], F32, name=f"vps{j}",
                                     tag=f"vps{j}") for j in range(2)]
                    wkv_all = wkvp.tile([P, KT, 2 * D], BF, tag="wkvall")
                    nc.sync.dma_start(wkv_all[:, 0:8, :], wkv3[:, 0:8, :])
                    nc.sync.dma_start(wkv_all[:, 8:20, :], wkv3[:, 8:20, :])
                    nc.sync.dma_start(wkv_all[:, 20:32, :], wkv3[:, 20:32, :])

                    hh0 = hidp.tile([P, 1, S], BF, name="hh0", tag="hh")
                    nc.sync.dma_start(hh0[:], hid3[:, 0:1, :])
                    hhs = []
                    for g in range(15):
                        hh = hidp.tile([P, 2, S], BF, tag="hh")
                        nc.sync.dma_start(hh[:],
                                          hid3[:, 1 + 2 * g:3 + 2 * g, :])
                        hhs.append(hh)
                        if g == 5:
                            # first 8 hh tiles (7 issues) fit the pool without
                            # buffer waits; slot wq4/wq5 in before the paced rest
                            w = wqp.tile([P, KT, P], BF, tag="w")
                            nc.sync.dma_start(w[:], wq4[4])
                            wq_pre[4] = w
                            w = wqp.tile([P, KT, P], BF, tag="w")
                            nc.sync.dma_start(w[:], wq4[5])
                            wq_pre[5] = w
                    hh31 = hidp.tile([P, 1, S], BF, name="hh31", tag="hh")
                    nc.sync.dma_start(hh31[:], hid3[:, 31:32, :])

                    def hh_slice(k):
                        if k == 0:
                            return hh0[:, 0, :]
                        if k == KT - 1:
                            return hh31[:, 0, :]
                        return hhs[(k - 1) // 2][:, (k - 1) % 2, :]

                    cosk_sb = constp.tile([ROT, S], BF, tag="ck")
                    nc.sync.dma_start(cosk_sb[:], cosk[:])
                    sink_sb = constp.tile([ROT, S], BF, tag="sk")
                    nc.sync.dma_start(sink_sb[:], sink[:])
                    bkv_sb = constp.tile([P, 2], F32, tag="bkv")
                    nc.sync.dma_start(bkv_sb[:], bkv2[:])
                    id_sb = constp.tile([P, P], BF, tag="id")
                    nc.sync.dma_start(id_sb[:], ident[:])
                    mask_sb = constp.tile([P, NKB, P], BF, tag="mask")
                    nc.sync.dma_start(mask_sb[:], maskt[:])
                    bo_sb = constp.tile([P, KT], F32, tag="bo")
                    nc.sync.dma_start(bo_sb[:], bo2[:])
                    w = wqp.tile([P, KT, P], BF, tag="w")
                    nc.sync.dma_start(w[:], wq4[6])
                    wq_pre[6] = w
                    w = wqp.tile([P, KT, P], BF, tag="w")
                    nc.sync.dma_start(w[:], wq4[7])
                    wq_pre[7] = w
                    ones_sb = constp.tile([P, 1], BF, tag="ones")
                    nc.gpsimd.memset(ones_sb[:], 1.0)

                    qrs[3] = emit_qproj_front(3, wq_pre.pop(3))
                    qrs[4] = emit_qproj_front(4, wq_pre.pop(4))

                    # KV matmuls (kps pair shares a weight load, then vps pair)
                    for k in range(KT):
                        st, sp_ = (k == 0), (k == KT - 1)
                        hhk = hh_slice(k)
                        for j in range(2):
                            nc.tensor.matmul(kps[j][:], wkv_all[:, k, 0:D],
                                             hhk[:, j * 512:(j + 1) * 512],
                                             start=st, stop=sp_)
                        for j in range(2):
                            nc.tensor.matmul(vps[j][:], wkv_all[:, k, D:2 * D],
                                             hhk[:, j * 512:(j + 1) * 512],
                                             start=st, stop=sp_)

                    # ---- KV epilogue: evictions, K RoPE, V PE-transpose ----
                    kfp = kvtmp.tile([P, S], BF, tag="kfp")
                    vbf = kvtmp.tile([P, S], BF, tag="vbf")
                    for j in range(2):
                        sl = slice(j * 512, (j + 1) * 512)
                        nc.scalar.activation(kfp[:, sl], kps[j][:], AF.Identity,
                                             bias=bkv_sb[:, 0:1])
                        nc.scalar.activation(vbf[:, sl], vps[j][:], AF.Identity,
                                             bias=bkv_sb[:, 1:2])
                    ksw = kvtmp.tile([ROT, S], BF, tag="ksw")
                    nc.scalar.dma_start(ksw[0:HR, :], kfp[HR:ROT, :])
                    nc.scalar.dma_start(ksw[HR:ROT, :], kfp[0:HR, :])
                    m1 = kvtmp.tile([ROT, S], BF, tag="m1")
                    m2 = kvtmp.tile([ROT, S], BF, tag="m2")
                    nc.vector.tensor_mul(m1[:], kfp[0:ROT, :], cosk_sb[:])
                    nc.vector.tensor_mul(m2[:], ksw[:], sink_sb[:])
                    nc.vector.tensor_sub(kbf[0:HR, :], m1[0:HR, :], m2[0:HR, :])
                    nc.vector.tensor_add(kbf[HR:ROT, :], m1[HR:ROT, :],
                                         m2[HR:ROT, :])
                    nc.vector.tensor_copy(kbf[ROT:P, :], kfp[ROT:P, :])
                    for t in range(NKB):
                        vt = pstr.tile([P, P], BF, tag="vt")
                        nc.tensor.transpose(vt[:], vbf[:, t * P:(t + 1) * P],
                                            id_sb[:])
                        nc.vector.tensor_copy(vnat[:, t, :], vt[:])

                with (
                    tc.tile_pool(name="pss", bufs=4, space="PSUM") as pss,
                    tc.tile_pool(name="pso", bufs=1, space="PSUM") as pso,
                    tc.tile_pool(name="pssum", bufs=1, space="PSUM") as pssum,
                ):
                    def attn_scores(qr, trange):
                        sps = []
                        for t in trange:
                            nv = NVMAX[t]
                            sp = pss.tile([P, NT], F32, tag="sp")
                            nc.tensor.matmul(sp[:, 0:nv],
                                             kbf[:, t * P:(t + 1) * P],
                                             qr[:, 0:nv], start=True, stop=True)
                            sps.append((t, sp))
                        for t, sp in sps:
                            lo, hi = MWIN[t]
                            nc.vector.tensor_add(sp[:, lo:hi], sp[:, lo:hi],
                                                 mask_sb[:, t, :])
                        return sps

                    def attn_pv(sps, op):
                        exs = []
                        for t, sp in sps:
                            ex = expp.tile([P, NT], BF, tag="ex")
                            nc.scalar.activation(ex[:, 0:NVMAX[t]],
                                                 sp[:, 0:NVMAX[t]], AF.Exp,
                                                 scale=SCALE)
                            exs.append((t, ex))
                        for t, ex in exs:
                            nc.tensor.matmul(op[:, 0:NVMAX[t]], vnat[:, t, :],
                                             ex[:, 0:NVMAX[t]],
                                             start=(t == 0), stop=(t == NKB - 1))
                        return exs

                    def attn_exacc(exacc, exs):
                        for t, ex in exs:
                            if t == 0:
                                nc.vector.tensor_copy(exacc[:], ex[:])
                            else:
                                nc.vector.tensor_add(exacc[:, 0:NVMAX[t]],
                                                     exacc[:, 0:NVMAX[t]],
                                                     ex[:, 0:NVMAX[t]])

                    def attn_end(h, op, exacc, pe_norm=False):
                        sums = pssum.tile([1, NT], F32, tag="sums")
                        nc.tensor.matmul(sums[:], ones_sb[:], exacc[:],
                                         start=True, stop=True)
                        rs = nrm.tile([1, NT], F32, tag="rs")
                        nc.vector.reciprocal_approx_fast(rs[:], sums[:])
                        if pe_norm:
                            # drain heads: broadcast 1/sums on the PE instead
                            # of gpsimd (whose op+drain is ~2us of exposed
                            # critical path once there is no filler left)
                            rsb = nrm.tile([1, NT], BF, tag="rsb")
                            nc.vector.tensor_copy(rsb[:], rs[:])
                            rb_ps = pssum.tile([P, NT], F32, tag="sums")
                            nc.tensor.matmul(rb_ps[:], ones_row[:], rsb[:],
                                             start=True, stop=True)
                            rb = nrm.tile([P, NT], F32, tag="rb")
                            nc.vector.tensor_copy(rb[:], rb_ps[:])
                        else:
                            rb = nrm.tile([P, NT], F32, tag="rb")
                            nc.gpsimd.partition_broadcast(rb[:], rs[:])
                        nc.vector.tensor_mul(attn_all[:, h, :], op[:], rb[:])

                    def emit_attn(h, qr, pe_norm=False):
                        op = pso.tile([P, NT], F32, tag="op")
                        exacc = exap.tile([P, NT], BF, tag="exacc")
                        sps1 = attn_scores(qr, range(0, 4))
                        exs1 = attn_pv(sps1, op)
                        attn_exacc(exacc, exs1)
                        sps2 = attn_scores(qr, range(4, NKB))
                        exs2 = attn_pv(sps2, op)
                        attn_exacc(exacc, exs2)
                        attn_end(h, op, exacc, pe_norm)

                    # ---- steady-state head pipeline (depth PRE=5) ----
                    pending = None   # (h, qp) awaiting evict+rope
                    for h in range(PRE, H):
                        if h in wq_pre:
                            wq_sb = wq_pre.pop(h)
                        else:
                            wq_sb = wqp.tile([P, KT, P], BF, tag="w")
                            nc.sync.dma_start(wq_sb[:], wq4[h])
                        qp = psq.tile([P, NT], F32, tag="qp")
                        ha = h - PRE
                        qr_a = qrs.pop(ha)
                        op = pso.tile([P, NT], F32, tag="op")
                        exacc = exap.tile([P, NT], BF, tag="exacc")
                        if pending is not None:
                            hp, qpp = pending
                            ev = qproj_evict(hp, qpp, nc.sync)
                        sps1 = attn_scores(qr_a, range(0, 4))
                        qproj_matmuls(qp, wq_sb, 0, 6)
                        exs1 = attn_pv(sps1, op)
                        qproj_matmuls(qp, wq_sb, 6, 12)
                        attn_exacc(exacc, exs1)
                        sps2 = attn_scores(qr_a, range(4, NKB))
                        exs2 = attn_pv(sps2, op)
                        qproj_matmuls(qp, wq_sb, 12, 16)
                        attn_exacc(exacc, exs2)
                        attn_end(ha, op, exacc)
                        if pending is not None:
                            qrs[pending[0]] = qproj_rope(*ev)
                        qproj_matmuls(qp, wq_sb, 16, KT)
                        pending = (h, qp)

                    # finish the last projected head
                    ev = qproj_evict(pending[0], pending[1], nc.sync)
                    qrs[pending[0]] = qproj_rope(*ev)

                    # ---- tail: drain 5 attentions, covered by O(1)/O(0)
                    # partial accumulations ----
                    emit_attn(H - 5, qrs.pop(H - 5))
                    wo_sb1 = wqp.tile([P, KT, P], BF, tag="w")
                    nc.sync.dma_start(wo_sb1[:], wo4[1])
                    fp1 = psq.tile([P, NT], F32, tag="qp")
                    for k in range(H - 5):
                        nc.tensor.matmul(fp1[:], wo_sb1[:, k, :],
                                         attn_all[:, k, :],
                                         start=(k == 0), stop=False)
                    emit_attn(H - 4, qrs.pop(H - 4))
                    wo_sb0 = wqp.tile([P, KT, P], BF, tag="w")
                    nc.sync.dma_start(wo_sb0[:], wo4[0])
                    fp0 = psq.tile([P, NT], F32, tag="qp")
                    for k in range(H - 5):
                        nc.tensor.matmul(fp0[:], wo_sb0[:, k, :],
                                         attn_all[:, k, :],
                                         start=(k == 0), stop=False)
                    emit_attn(H - 3, qrs.pop(H - 3))
                    nc.tensor.matmul(fp1[:], wo_sb1[:, H - 5, :],
                                     attn_all[:, H - 5, :],
                                     start=False, stop=False)
                    nc.tensor.matmul(fp0[:], wo_sb0[:, H - 5, :],
                                     attn_all[:, H - 5, :],
                                     start=False, stop=False)
                    emit_attn(H - 2, qrs.pop(H - 2), pe_norm=True)
                    nc.tensor.matmul(fp1[:], wo_sb1[:, H - 4, :],
                                     attn_all[:, H - 4, :],
                                     start=False, stop=False)
                    nc.tensor.matmul(fp0[:], wo_sb0[:, H - 4, :],
                                     attn_all[:, H - 4, :],
                                     start=False, stop=False)
                    emit_attn(H - 1, qrs.pop(H - 1), pe_norm=True)
                    for k in range(H - 3, KT - 1):
                        nc.tensor.matmul(fp1[:], wo_sb1[:, k, :],
                                         attn_all[:, k, :],
                                         start=False, stop=False)
                        nc.tensor.matmul(fp0[:], wo_sb0[:, k, :],
                                         attn_all[:, k, :],
                                         start=False, stop=False)
                    nc.tensor.matmul(fp1[:], wo_sb1[:, KT - 1, :],
                                     attn_all[:, KT - 1, :],
                                     start=False, stop=True)
                    ob1 = outsb.tile([P, NT], BF, tag="ob")
                    nc.scalar.activation(ob1[:], fp1[:], AF.Identity,
                                         bias=bo_sb[:, 1:2])
                    nc.sync.dma_start(outp[1], ob1[:])
                    nc.tensor.matmul(fp0[:], wo_sb0[:, KT - 1, :],
                                     attn_all[:, KT - 1, :],
                                     start=False, stop=True)
                    ob0 = outsb.tile([P, NT], BF, tag="ob")
                    nc.scalar.activation(ob0[:], fp0[:], AF.Identity,
                                         bias=bo_sb[:, 0:1])
                    nc.sync.dma_start(outp[0], ob0[:])

                    # ---- output projection (rest) ----
                    for m in range(2, KT):
                        wo_sb = wqp.tile([P, KT, P], BF, tag="w")
                        nc.sync.dma_start(wo_sb[:], wo4[m])
                        fp = psq.tile([P, NT], F32, tag="qp")
                        for k in range(KT):
                            nc.tensor.matmul(fp[:], wo_sb[:, k, :],
                                             attn_all[:, k, :],
                                             start=(k == 0), stop=(k == KT - 1))
                        ob = outsb.tile([P, NT], BF, tag="ob")
                        if m == KT - 1:
                            # halve the final eviction->writeback drain
                            nc.scalar.activation(ob[:, 0:NT // 2],
                                                 fp[:, 0:NT // 2], AF.Identity,
                                                 bias=bo_sb[:, m:m + 1])
                            nc.sync.dma_start(outp[m, :, 0:NT // 2],
                                              ob[:, 0:NT // 2])
                            nc.scalar.activation(ob[:, NT // 2:NT],
                                                 fp[:, NT // 2:NT], AF.Identity,
                                                 bias=bo_sb[:, m:m + 1])
                            nc.sync.dma_start(outp[m, :, NT // 2:NT],
                                              ob[:, NT // 2:NT])
                        else:
                            nc.scalar.activation(ob[:], fp[:], AF.Identity,
                                                 bias=bo_sb[:, m:m + 1])
                            nc.sync.dma_start(outp[m], ob[:])

    nc.compile()
    return nc


def _get_prog():
    global _PROG
    if _PROG is None:
        _PROG = _build()
    return _PROG


def _qcols(hf):
    return np.concatenate([np.arange(b * P, (b + 1) * P) for b in BLOCKS[hf]])


def _prepare_inmaps(inputs):
    pos = np.asarray(inputs["position_ids"])
    hs = np.asarray(inputs["hidden_states"], np.float32)
    Wq = np.asarray(inputs["Wq"], np.float32)
    bq = np.asarray(inputs["bq"], np.float32)
    Wkv = np.asarray(inputs["Wkv"], np.float32)
    bkv = np.asarray(inputs["bkv"], np.float32)
    Wo = np.asarray(inputs["Wo"], np.float32)
    bo = np.asarray(inputs["bo"], np.float32)

    shared = {
        "wq4": np.ascontiguousarray(
            Wq.reshape(H, P, KT, P).transpose(0, 3, 2, 1)).astype(bf16),
        "wo4": np.ascontiguousarray(
            Wo.reshape(KT, P, KT, P).transpose(0, 3, 2, 1)).astype(bf16),
        "wkv3": np.ascontiguousarray(
            Wkv.T.reshape(KT, P, 2 * D).transpose(1, 0, 2)).astype(bf16),
        "bq2": np.ascontiguousarray(bq.reshape(H, P).T),
        "bo2": np.ascontiguousarray(bo.reshape(KT, P).T),
        "bkv2": np.ascontiguousarray(bkv.reshape(2, P).T),
        "ident": np.eye(P, dtype=np.float32).astype(bf16),
    }

    invf = (1.0 / (ROPE_BASE ** (np.arange(0, ROT, 2, dtype=np.float32)
                                 / np.float32(ROT)))).astype(np.float32)
    in_maps = []
    for c in range(NCORES):
        b, hf = c // 2, c % 2
        qc = _qcols(hf)
        posb = pos[b].astype(np.float32)
        ang = invf[:, None] * posb[None, :]          # [32, S]
        cos1 = np.cos(ang).astype(np.float32)
        sin1 = np.sin(ang).astype(np.float32)
        cos2k = np.concatenate([cos1, cos1], 0)      # [64, S]
        sin2k = np.concatenate([sin1, sin1], 0)
        hidT = np.ascontiguousarray(hs[b].T)         # [HID, S]
        kpos = (np.arange(NKB)[None, :, None] * P
                + np.arange(P)[:, None, None])       # [P, NKB, 1]
        # Causal mask is over sequence INDICES (jnp.tril in the reference),
        # not position values; qc are the packed columns' sequence indices.
        mask = np.where(kpos <= qc[None, None, :], 0.0, -1e30).astype(np.float32)
        for t in range(NKB):
            lo, hi = MWIN[t]
            assert not mask[:, t, :lo].any() and not mask[:, t, hi:NVMAX[t]].any(), \
                f"mask outside window at t={t}"
        maskc = np.stack([mask[:, t, MWIN[t][0]:MWIN[t][1]]
                          for t in range(NKB)], axis=1)   # [P, NKB, 128]
        m = dict(shared)
        m["hid3"] = np.ascontiguousarray(
            hidT.reshape(KT, P, S).transpose(1, 0, 2)).astype(bf16)
        m["hidq"] = np.ascontiguousarray(
            hidT[:, qc].reshape(KT, P, NT).transpose(1, 0, 2)).astype(bf16)
        m["cosq"] = np.ascontiguousarray(cos2k[:, qc]).astype(bf16)
        m["sinq"] = np.ascontiguousarray(sin2k[:, qc]).astype(bf16)
        m["cosk"] = cos2k.astype(bf16)
        m["sink"] = sin2k.astype(bf16)
        m["maskt"] = np.ascontiguousarray(maskc).astype(bf16)
        in_maps.append(m)
    return in_maps


def _assemble(results):
    out = np.empty((B, S, HID), np.float32)
    for c in range(NCORES):
        b, hf = c // 2, c % 2
        outT = np.asarray(results[c]["out"], np.float32).reshape(HID, NT)
        out[b, _qcols(hf), :] = outT.T
    return out


def _run(inputs, trace=False, **kw):
    nc = _get_prog()
    in_maps = _prepare_inmaps(inputs)
    try:
        res = run_bass_kernel_spmd(nc, in_maps, list(range(NCORES)),
                                   trace=trace, **kw)
    except Exception:
        # transient device wedge (e.g. NRT_EXEC_UNIT_UNRECOVERABLE) — retry once
        res = run_bass_kernel_spmd(nc, in_maps, list(range(NCORES)),
                                   trace=trace, **kw)
    return _assemble(res.results), res


def kernel(**inputs):
    out, _ = _run(inputs)
    return out
